# revision 10
# baseline (speedup 1.0000x reference)
"""Trainium2 Bass kernel for nn_CrossedAttention (B=2, NQ=NK=8192, C=256, C4=64).

Hybrid batch x sequence sharding across 8 NeuronCores: core i handles batch
i//4 and q rows (i%4)*2048..(i%4+1)*2048 of that batch (b=1, nqs=2048 per
core). Each core holds the kv_tensor of its batch only.

Host-side staging: kv/q are pre-cast to bf16 and pre-transposed to
channel-on-partition halves ([1, 2, 128, n]); weights are pre-transposed,
the BatchNorm scale A is folded into wt, and the remaining affine offset
B2 is shipped as a row-replicated constant.

Per-core pipeline (software-pipelined via an explicit event queue):
  1. project: x_kT = wk @ kvT and x_qT = wq @ qT (fp8e4m3, duplicated
     onto partitions 64-127 for PE row tiling), x_v = kv @ wv^T with an
     appended ones-column (fp8, natural [k, c]). Evictions split DVE/ACT.
  2. energyT[k,q] = x_kT.T @ x_qT per 2-chunk duo; the two chunks run as
     concurrent K=64 matmuls in PE row groups (0,0)/(64,0) into
     double-buffered 2-bank PSUM tiles. One contiguous FD=1024 ACT exp
     per duo -> fp8 attT group tiles. No max-subtraction: |energy| <~ 6.
  3. PV: per 128-q slab accumulate DoubleRow K=256 fp8 matmuls
     (attT chunk-pair stationary, [x_v|ones] chunk-pair moving) over 32
     pairs -> unnormalized x_r plus the softmax denominator in one PSUM
     bank. PV matmuls and the epilogue stages of group g are interleaved
     at ~4-matmul granularity with the energy duos of group g+1 via the
     event queue, so the in-order PE queue never head-of-line blocks on
     the DVE epilogue chain.
  4. epilogue per slab, split into stages E1 (DVE: recip, xr, res),
     E2a (PE transposes + DVE eviction), E2c (PE: y = res @ (wt*A)^T),
     E3 (DVE: +B2, relu, +q residual, DMA out), each emitted a few
     queue slots after its producer so no engine stalls another.
"""

import numpy as np
import ml_dtypes

import concourse.bass as bass
import concourse.mybir as mybir
import concourse.tile as tile
from concourse import bacc, bass_utils
from concourse.masks import make_identity

F32 = mybir.dt.float32
BF16 = mybir.dt.bfloat16
FP8 = mybir.dt.float8e4
AF = mybir.ActivationFunctionType

# dtype for the attention weights (exp output) and x_v in the PV matmul.
ATT_DT = FP8

# timing ablation: "" (full), "preproc_only", "no_pv", "energy_only",
# "no_epi" — timing builds only, outputs are garbage for non-empty values.
import os as _os
ABLATE = _os.environ.get("BASS_ABLATE", "")

C = 256
C4 = 64
B = 2
NQ = 8192
NK = 8192
N_CORES = 8
BN_EPS = 1e-5

B_PER_CORE = 1
SEQ_SHARDS = N_CORES // B          # 4
NQS = NQ // SEQ_SHARDS             # 2048 q rows per core


def build_nc(b=B_PER_CORE, nqs=NQS, nk=NK, reps=1):
    """Build the per-core Bass module. nqs = q rows per core per batch.

    reps>1 wraps the whole workload in an on-device For_i loop — used only
    for timing (amortizes host dispatch overhead); results are idempotent.
    """
    nc = bacc.Bacc("TRN2", target_bir_lowering=False, debug=False)

    q = nc.dram_tensor("q", [b, nqs, C], F32, kind="ExternalInput").ap()
    # kv/q pre-transposed to [128, 2, n] fp8: partition p, half h hold
    # channel p + 128h — the DoubleRow (Ki, Ko=2, dim) stationary layout.
    kvt_d = nc.dram_tensor("kvt_in", [b, 128, 2, nk], FP8, kind="ExternalInput").ap()
    qt_d = nc.dram_tensor("qt_in", [b, 128, 2, nqs], FP8, kind="ExternalInput").ap()
    wq_d = nc.dram_tensor("wq_t", [128, 2, C4], FP8, kind="ExternalInput").ap()
    wk_d = nc.dram_tensor("wk_t", [128, 2, C4], FP8, kind="ExternalInput").ap()
    wv_d = nc.dram_tensor("wv_t", [128, 2, C], FP8, kind="ExternalInput").ap()
    wt_d = nc.dram_tensor("wt_t", [2, 128, C], F32, kind="ExternalInput").ap()
    b_d = nc.dram_tensor("b_rep", [128, C], F32, kind="ExternalInput").ap()
    out = nc.dram_tensor("out", [b, nqs, C], F32, kind="ExternalOutput").ap()

    KC = nk // 128          # number of 128-row kv chunks
    GQ = min(512, nqs)      # q rows per energy group
    NG = nqs // GQ
    SLABS = GQ // 128       # q slabs per group
    SEG = 8 if nk >= 4096 else 1    # kv staging segments

    with tile.TileContext(nc) as tc:
        with (
            tc.tile_pool(name="const", bufs=1) as constp,
            tc.tile_pool(name="kvtp", bufs=2) as kvtp,
            tc.tile_pool(name="attp", bufs=2) as attp,
            tc.tile_pool(name="xvp", bufs=2) as xvp,
            tc.tile_pool(name="xkp", bufs=2) as xkp,
            tc.tile_pool(name="xqp", bufs=2) as xqp,
            tc.tile_pool(name="qtp", bufs=2) as qtp,
            tc.tile_pool(name="workp", bufs=3) as workp,
            tc.tile_pool(name="enps", bufs=2, space="PSUM") as enps,
            tc.tile_pool(name="pvps", bufs=2, space="PSUM") as pvps,
            tc.tile_pool(name="mmps", bufs=2, space="PSUM") as mmps,
        ):
            # ---- constants ----
            ident = constp.tile([128, 128], F32)
            make_identity(nc, ident)
            wq_sb = constp.tile([128, 2, C4], FP8)
            wk_sb = constp.tile([128, 2, C4], FP8)
            wv_sb = constp.tile([128, 2, C], FP8)
            wt_sb = constp.tile([128, 2, C], F32)
            b_sb = constp.tile([128, C], F32)
            nc.sync.dma_start(wq_sb, wq_d)
            nc.sync.dma_start(wk_sb, wk_d)
            nc.sync.dma_start(wv_sb, wv_d)
            for h in range(2):
                nc.sync.dma_start(wt_sb[:, h], wt_d[h])
            nc.sync.dma_start(b_sb, b_d)

            def body(_it=None):
                emit_body(
                    nc, tc, b, nqs, nk,
                    q, kvt_d, qt_d, out,
                    ident, wq_sb, wk_sb, wv_sb, wt_sb, b_sb,
                    kvtp, attp, xvp, xkp, xqp, qtp, workp,
                    enps, pvps, mmps,
                    KC, GQ, NG, SLABS, SEG,
                )

            if reps == 1:
                body()
            else:
                with tc.For_i(0, reps, 1) as _it:
                    body(_it)
    nc.compile()
    return nc


def emit_body(nc, tc, b, nqs, nk, q, kvt_d, qt_d, out,
              ident, wq_sb, wk_sb, wv_sb, wt_sb, b_sb,
              kvtp, attp, xvp, xkp, xqp, qtp, workp,
              enps, pvps, mmps,
              KC, GQ, NG, SLABS, SEG):

    DUOS = KC // 2
    PVB = 8                 # PV matmul batches per slab
    PVM = (KC // 2) // PVB  # DoubleRow matmuls per batch (4)

    pend = []               # queued emission items (closures)

    def pump(n):
        for _ in range(n):
            if not pend:
                return
            pend.pop(0)()

    def enqueue_slab(bi, g, pair, xv):
        """Queue the PV + epilogue items for all slabs of (bi, g)."""
        states = [dict() for _ in range(SLABS)]

        def pvb(s, k):
            def run():
                st = states[s]
                if k == 0:
                    row0 = g * GQ + s * 128
                    st["row0"] = row0
                    qn = workp.tile([128, C], F32, name="qn", tag="qn")
                    nc.scalar.dma_start(qn, q[bi, row0 : row0 + 128])
                    st["qn"] = qn
                    st["pv"] = pvps.tile([128, 512], F32, name="pv", tag="pv")
                pv = st["pv"]
                qoff = s * 128
                for jp in range(k * PVM, (k + 1) * PVM):
                    nc.tensor.matmul(
                        pv[:, : C + 2],
                        pair[:, 2 * jp : 2 * jp + 2, qoff : qoff + 128],
                        xv[:, 2 * jp : 2 * jp + 2, : C + 2],
                        start=(jp == 0),
                        stop=(jp == KC // 2 - 1),
                        perf_mode=mybir.MatmulPerfMode.DoubleRow,
                    )
            return run

        def e1(s):
            def run():
                st = states[s]
                pv = st["pv"]
                if ABLATE == "no_epi":
                    ot0 = workp.tile([128, C], F32, name="ot0", tag="ot")
                    nc.vector.tensor_copy(ot0, pv[:, :C])
                    nc.scalar.dma_start(
                        out[bi, st["row0"] : st["row0"] + 128], ot0)
                    return
                rden = workp.tile([128, 1], F32, name="rden", tag="rden")
                nc.vector.reciprocal(rden, pv[:, C : C + 1])
                xr = workp.tile([128, C], F32, name="xr", tag="xr")
                nc.vector.tensor_scalar_mul(xr, pv[:, :C], rden)
                res = workp.tile([128, C], F32, name="res", tag="res")
                nc.vector.tensor_sub(res, st["qn"], xr)
                st["res"] = res
            return run

        def e2a(s):
            def run():
                if ABLATE == "no_epi":
                    return
                st = states[s]
                res = st["res"]
                tp = mmps.tile([128, 512], F32, name="tp", tag="mm")
                nc.tensor.transpose(tp[:, 0:128], res[:, 0:128], ident)
                nc.tensor.transpose(tp[:, 128:256], res[:, 128:256], ident)
                rest = workp.tile([128, C], F32, name="rest", tag="rest")
                nc.vector.tensor_copy(rest, tp[:, :C])
                st["rest"] = rest
            return run

        def e2c(s):
            def run():
                if ABLATE == "no_epi":
                    return
                st = states[s]
                rest = st["rest"]
                yp = mmps.tile([128, 512], F32, name="yp", tag="mm")
                for h in range(2):
                    nc.tensor.matmul(
                        yp[:, :C],
                        rest[:, h * 128 : (h + 1) * 128],
                        wt_sb[:, h],
                        start=(h == 0),
                        stop=(h == 1),
                    )
                st["yp"] = yp
            return run

        def e3(s):
            def run():
                if ABLATE == "no_epi":
                    return
                st = states[s]
                # out = relu(y + B2) + q   (A already folded into wt)
                t1 = workp.tile([128, C], F32, name="t1", tag="t1")
                nc.vector.tensor_add(t1, st["yp"][:, :C], b_sb)
                nc.vector.tensor_scalar_max(t1, t1, 0.0)
                ot = workp.tile([128, C], F32, name="ot", tag="ot")
                nc.vector.tensor_add(ot, t1, st["qn"])
                nc.scalar.dma_start(out[bi, st["row0"] : st["row0"] + 128], ot)
            return run

        # Interleaved item order: each slab's epilogue stages are spaced
        # out between the next slab's PV batches so producer->consumer
        # chains never stall the emitting engine's in-order queue.
        for s in range(SLABS):
            pend.extend(pvb(s, k) for k in range(0, 4))
            if s > 0:
                pend.append(e2a(s - 1))
            pend.extend(pvb(s, k) for k in range(4, 6))
            if s > 0:
                pend.append(e2c(s - 1))
                pend.append(e3(s - 1))
            pend.extend(pvb(s, k) for k in range(6, PVB))
            pend.append(e1(s))
        pend.append(e2a(SLABS - 1))
        pend.append(e2c(SLABS - 1))
        pend.append(e3(SLABS - 1))

    if ABLATE == "preproc_only":
        for bi in range(b):
            nc.sync.dma_start(out[bi], q[bi])

    for bi in range(b):
        # ---- load host-pretransposed kvT/qT (fp8, DoubleRow layout) ----
        kvt8 = kvtp.tile([128, 2, nk], FP8, name=f"kvt{bi}", tag="kvt")
        for sg in range(SEG):
            r0, r1 = sg * (nk // SEG), (sg + 1) * (nk // SEG)
            nc.sync.dma_start(kvt8[:, :, r0:r1], kvt_d[bi, :, :, r0:r1])
        qt8 = qtp.tile([128, 2, nqs], FP8, name=f"qt{bi}", tag="qt")
        nc.sync.dma_start(qt8, qt_d[bi])

        # ---- x_kT [C4, nk] (fp8), duplicated to partitions 64-127
        # so energy chunk pairs can row-tile the PE array.  One K=256
        # DoubleRow matmul per 512 columns. ----
        xkt = xkp.tile([128, nk], ATT_DT, name=f"xkt{bi}", tag="xkt")
        for ji, j0 in enumerate(range(0, nk, 512)):
            w = min(512, nk - j0)
            ps_k = mmps.tile([128, 512], F32, name="ps_k", tag="mm")
            nc.tensor.matmul(
                ps_k[:C4, :w],
                wk_sb,
                kvt8[:, :, j0 : j0 + w],
                start=True,
                stop=True,
                perf_mode=mybir.MatmulPerfMode.DoubleRow,
            )
            if ji % 2 == 0:
                nc.vector.tensor_copy(xkt[:C4, j0 : j0 + w], ps_k[:C4, :w])
            else:
                nc.scalar.copy(xkt[:C4, j0 : j0 + w], ps_k[:C4, :w])
        for sg in range(SEG):
            r0, r1 = sg * (nk // SEG), (sg + 1) * (nk // SEG)
            nc.sync.dma_start(xkt[C4:128, r0:r1], xkt[:C4, r0:r1])

        # ---- x_qT [C4, nqs] (fp8), duplicated likewise ----
        xqt = xqp.tile([128, nqs], ATT_DT, name=f"xqt{bi}", tag="xqt")
        for j0 in range(0, nqs, 512):
            w = min(512, nqs - j0)
            ps_q = mmps.tile([128, 512], F32, name="ps_q", tag="mm")
            nc.tensor.matmul(
                ps_q[:C4, :w],
                wq_sb,
                qt8[:, :, j0 : j0 + w],
                start=True,
                stop=True,
                perf_mode=mybir.MatmulPerfMode.DoubleRow,
            )
            nc.vector.tensor_copy(xqt[:C4, j0 : j0 + w], ps_q[:C4, :w])
        nc.sync.dma_start(xqt[C4:128, :], xqt[:C4, :])

        # ---- x_v [k, C] + ones column (fp8, natural layout) ----
        # inner dim padded to 272 so the chunk-pair stride is a multiple
        # of 16B (DoubleRow AP constraint); evictions alternate DVE/ACT.
        # One K=256 DoubleRow matmul per kv chunk (kv chunk stationary).
        xv = xvp.tile([128, KC, C + 16], ATT_DT, name=f"xv{bi}", tag="xv")
        nc.vector.memset(xv[:, :, C : C + 2], 1.0)
        for jp in range(KC // 2):
            ps_v = mmps.tile([128, 512], F32, name="ps_v", tag="mm")
            for jj in range(2):
                j = jp * 2 + jj
                nc.tensor.matmul(
                    ps_v[:, jj * 256 : jj * 256 + C],
                    kvt8[:, :, j * 128 : (j + 1) * 128],
                    wv_sb,
                    start=True,
                    stop=True,
                    perf_mode=mybir.MatmulPerfMode.DoubleRow,
                )
            dst = xv[:, jp * 2 : jp * 2 + 2, :C]
            src = ps_v.rearrange("p (a c) -> p a c", a=2)
            if jp % 2 == 0:
                nc.vector.tensor_copy(dst, src)
            else:
                nc.scalar.copy(dst, src)

        if ABLATE == "preproc_only":
            continue

        # ---- attention groups (energy/exp interleaved with prior-group
        # PV + epilogue items via the event queue) ----
        for g in range(NG):
            q0 = g * GQ
            energy_only = ABLATE == "energy_only"
            pair = (
                attp.tile([128, KC, GQ], ATT_DT, name=f"att{bi}{g}", tag="att")
                if not energy_only else None
            )
            done = 0
            for dd in range(DUOS):
                # smooth pacing: drain queued items evenly across duos
                quota = (len(pend) + DUOS - dd - 1) // (DUOS - dd) if pend else 0
                enp = enps.tile([128, 2, GQ], F32, name="enp", tag="en")
                j = dd * 2
                # two K=64 matmuls run concurrently in PE row groups
                # (0,0) / (64,0) via the duplicated partition halves
                nc.tensor.matmul(
                    enp[:, 0],
                    xkt[:C4, j * 128 : (j + 1) * 128],
                    xqt[:C4, q0 : q0 + GQ],
                    start=True,
                    stop=True,
                    tile_position=(0, 0),
                )
                nc.tensor.matmul(
                    enp[:, 1],
                    xkt[C4:128, (j + 1) * 128 : (j + 2) * 128],
                    xqt[C4:128, q0 : q0 + GQ],
                    start=True,
                    stop=True,
                    tile_position=(64, 0),
                )
                if energy_only:
                    if dd == DUOS - 1:
                        zz = workp.tile([128, 2], F32, name="zz", tag="zz")
                        nc.vector.tensor_copy(zz, enp[:, :, 0:1])
                    continue
                nc.scalar.activation(pair[:, j : j + 2, :], enp, AF.Exp)
                pump(quota)
            if energy_only:
                if g == 0:
                    nc.sync.dma_start(out[bi], q[bi])
                continue
            if ABLATE == "no_pv":
                if g == 0:
                    nc.sync.dma_start(out[bi], q[bi])
                continue
            enqueue_slab(bi, g, pair, xv)

    # drain the final group's PV + epilogue
    while pend:
        pump(1)


FP8_NP = ml_dtypes.float8_e4m3


def _host_consts(wq, wk, wv, wt, bt, gamma, beta, run_mean, run_var):
    """Precompute weight layouts + folded BN affine on the host."""

    def chunks_t(w):
        # w [d, C] -> w.T [C, d] -> [2, 128, d]
        wT = np.ascontiguousarray(w.T.astype(np.float32))
        return wT.reshape(2, 128, -1)

    def chunks_t8(w):
        # w [d, C] -> [128, 2, d] fp8: partition p, half h = channel p+128h
        wT = np.ascontiguousarray(w.T.astype(np.float32))
        return np.ascontiguousarray(
            wT.reshape(2, 128, -1).transpose(1, 0, 2)).astype(FP8_NP)

    a = (gamma / np.sqrt(run_var + BN_EPS)).astype(np.float32)
    b2 = ((bt - run_mean) * a + beta).astype(np.float32)
    wt_scaled = (wt.astype(np.float32) * a[:, None]).astype(np.float32)
    return {
        "wq_t": chunks_t8(wq),
        "wk_t": chunks_t8(wk),
        "wv_t": chunks_t8(wv),
        "wt_t": chunks_t(wt_scaled).astype(np.float32),
        "b_rep": np.tile(b2[None, :], (128, 1)),
    }


def _host_transpose(x):
    """[b, n, C] f32 -> [b, 128, 2, n] fp8 (DoubleRow channel layout)."""
    b, n, _ = x.shape
    xt = x.transpose(0, 2, 1).reshape(b, 2, 128, n).transpose(0, 2, 1, 3)
    return np.ascontiguousarray(xt).astype(FP8_NP)


def make_in_maps(q_tensor, kv_tensor, consts, n_cores=N_CORES):
    """Hybrid shard: core i -> batch i//SEQ_SHARDS, seq shard i%SEQ_SHARDS.
    Every core gets the (pre-transposed) kv of its batch only."""
    b, nq, _ = q_tensor.shape
    seq_shards = n_cores // b
    nqs = nq // seq_shards
    kvts = [_host_transpose(kv_tensor[bi : bi + 1]) for bi in range(b)]
    in_maps = []
    for i in range(n_cores):
        bi, si = divmod(i, seq_shards)
        qs = np.ascontiguousarray(q_tensor[bi : bi + 1, si * nqs : (si + 1) * nqs])
        m = dict(consts)
        m["q"] = qs
        m["qt_in"] = _host_transpose(qs)
        m["kvt_in"] = kvts[bi]
        in_maps.append(m)
    return in_maps


def assemble_full(per_core, b=B, nq=NQ, n_cores=N_CORES):
    """Inverse of make_in_maps for the outputs: [n_cores][1, nqs, C] -> [b, nq, C]."""
    seq_shards = n_cores // b
    nqs = nq // seq_shards
    out = np.empty((b, nq, C), dtype=np.float32)
    for i in range(n_cores):
        bi, si = divmod(i, seq_shards)
        out[bi, si * nqs : (si + 1) * nqs] = per_core[i][0]
    return out


_NC_CACHE = {}


def _get_nc(b, nqs, nk):
    key = (b, nqs, nk)
    if key not in _NC_CACHE:
        _NC_CACHE[key] = build_nc(b, nqs, nk)
    return _NC_CACHE[key]


def kernel(q_tensor, kv_tensor, wq, wk, wv, wt, bt, gamma, beta, run_mean, run_var):
    q_tensor = np.asarray(q_tensor, dtype=np.float32)
    kv_tensor = np.asarray(kv_tensor, dtype=np.float32)
    consts = _host_consts(
        np.asarray(wq), np.asarray(wk), np.asarray(wv), np.asarray(wt),
        np.asarray(bt), np.asarray(gamma), np.asarray(beta),
        np.asarray(run_mean), np.asarray(run_var),
    )

    b, nq, _ = q_tensor.shape
    nk = kv_tensor.shape[1]
    nqs = nq // (N_CORES // b)
    nc = _get_nc(1, nqs, nk)

    in_maps = make_in_maps(q_tensor, kv_tensor, consts)

    res = bass_utils.run_bass_kernel_spmd(nc, in_maps, core_ids=list(range(N_CORES)))
    return assemble_full([res.results[i]["out"] for i in range(N_CORES)], b, nq)
